# revision 32
# baseline (speedup 1.0000x reference)
"""Masked linear (CantorLinear): y = x @ (weight*mask).T + bias.

Structure exploited: the Cantor mask keeps ~3.9% of weights, arranged as 256
contiguous runs in the flattened (out, in) index space. Only 240 of the 2048
output rows have any nonzero weight. The kernel packs those rows into a
[256, 2048] compact weight, computes the compact matmul on 8 NeuronCores
(data-parallel over the 16384 sequence positions), and scatters the 240
computed columns into a bias-broadcast full output on the host. The other
1808 output columns are exactly bias (filled host-side in fp32).

Device kernel (per core), mode e3w16 (default): x streams as fp8-e3m4
(moving operand) while w stays fp16 (stationary) — a MIXED-dtype matmul,
which HW supports (only fp32 must pair with fp32). e3m4's 4-bit mantissa
keeps the x-quantization error at 1.19e-2 max-rel (gate 2e-2; e4m3 measures
2.34e-2 and fails), fp16 w contributes ~nothing, and fp8 x halves HBM
traffic (4.2MB x + 1MB y fp16 per iteration ~ 14us DMA vs 26.6us PE).
x is pre-scaled by 2 (max |2x| ~ 11.2 < e3m4 max 15.5) and the 1/2 descale
is folded into the fp16 weights host-side so evictions are pure bias-adds
that can alternate between the scalar and vector engines.

Loop order "kouter" (default): all 4 sequence tiles' PSUMs (2 row-blocks x
4 si = 8 PSUM banks) accumulate together with k outermost, so each
stationary weight tile is reused across 4 consecutive matmuls; this
measured ~28us mean vs 33.0us for the per-si loop (device has several-us
run-to-run noise). NT=512 (full PSUM bank per tile). On top, rows are
sorted so the second 128-row PSUM block only touches 13 of 16 k-subtiles
(CANTOR_BSPARSE=1 default; 29/32 matmul groups, shared k-subtiles run
first so the narrow block's PSUMs retire early): 24.7/26.2us measured.
fp8dr (e4m3 DoubleRow, fails the error gate), fp16 (baseline), e3, bf16
paths are kept for A/B via CANTOR_MM_MODE.
"""

import os
import numpy as np

import concourse.bacc as bacc
import concourse.mybir as mybir
import concourse.tile as tile
from concourse.bass_utils import run_bass_kernel_spmd

B, SQ = 4, 4096
IN_F = 2048
OUT_F = 2048
S = B * SQ                 # 16384 flattened sequence positions
NCORES = 8
S_SH = S // NCORES         # 2048 per core
R_PAD = 256                # compact out-rows padded (240 real)
P = 128
KS = IN_F // P             # 16 k-subtiles (fp16 path)
KD = IN_F // (2 * P)       # 8 double-row k-pair blocks (fp8dr path)
NT = int(os.environ.get("CANTOR_NT", "512"))   # sequence-tile width
MB = R_PAD // P            # 2 output partition blocks

SX = 32.0                  # e4m3 pre-scale for x (fp8dr)
SW = 8192.0                # e4m3 pre-scale for w (fp8dr)
DESCALE = 1.0 / (SX * SW)
SX3 = 2.0                  # e3m4 pre-scale for x (max |2x| ~ 11.2 < 15.5)
SW3 = 512.0                # e3m4 pre-scale for w (e3 mode)

# matmul input dtype: "e3w16" (default: x e3m4 + w fp16), "fp8dr", "fp16",
# "e3", "bf16", "f32r", "f32"
MM_MODE = os.environ.get("CANTOR_MM_MODE", "e3w16")
# loop order: "kouter" reuses each stationary weight tile across all 4
# sequence tiles (8 live PSUM banks), amortizing PE weight-load overhead;
# "siouter" is the original per-si loop.
LOOP = os.environ.get("CANTOR_LOOP", "kouter")
OUT_FP16 = os.environ.get("CANTOR_OUT_FP16", "1") == "1"
# send the e4m3 residual of w as a second accumulation pass (fp8dr only)
WCOMP = os.environ.get("CANTOR_WCOMP", "1") == "1"
# repeat the whole kernel body LOOPS times inside one NEFF (benchmarking only)
LOOPS = int(os.environ.get("CANTOR_BENCH_LOOPS", "1"))

LAST_RESULTS = None  # BassKernelResults of the most recent run (for test.py)

_NC_CACHE = {}


def _build_nc_fp8dr(loops: int):
    io_dt = mybir.dt.float8e4
    y_dt = mybir.dt.float16 if OUT_FP16 else mybir.dt.float32
    KK = 2 * KD if WCOMP else KD    # w copies: [w8, rw8] or just w8
    nc = bacc.Bacc("TRN2", target_bir_lowering=False, debug=False)
    n_si = S_SH // NT
    xt = nc.dram_tensor("xt", [n_si, P, KD, 2, NT], io_dt, kind="ExternalInput")
    wt = nc.dram_tensor("wt", [P, KK, 2, R_PAD], io_dt, kind="ExternalInput")
    bc = nc.dram_tensor("bc", [R_PAD], mybir.dt.float32, kind="ExternalInput")
    yt = nc.dram_tensor("yt", [R_PAD, S_SH], y_dt, kind="ExternalOutput")

    bc_r = bc.rearrange("(m p) -> p m", p=P)

    with tile.TileContext(nc) as tc:
        with (
            tc.tile_pool(name="wpool", bufs=1) as wpool,
            tc.tile_pool(name="xpool", bufs=int(os.environ.get("CANTOR_XBUFS", "4"))) as xpool,
            tc.tile_pool(name="opool", bufs=int(os.environ.get("CANTOR_OBUFS", "4"))) as opool,
            tc.tile_pool(name="pspool", bufs=int(os.environ.get("CANTOR_PSBUFS", "4")), space="PSUM") as pspool,
        ):
            w_sb = wpool.tile([P, KK, 2, R_PAD], io_dt)
            nc.sync.dma_start(w_sb[:], wt[:])
            b_sb = wpool.tile([P, MB], mybir.dt.float32)
            nc.sync.dma_start(b_sb[:], bc_r)

            def body(_i=None):
                for si in range(S_SH // NT):
                    x_sb = xpool.tile([P, KD, 2, NT], io_dt, tag="xld")
                    nc.sync.dma_start(x_sb[:], xt[si])
                    for m in range(MB):
                        ps = pspool.tile([P, NT], mybir.dt.float32, tag="ps")
                        for kk in range(KK):
                            nc.tensor.matmul(
                                ps[:],
                                lhsT=w_sb[:, kk, :, m * P:(m + 1) * P],
                                rhs=x_sb[:, kk % KD, :, :],
                                start=(kk == 0),
                                stop=(kk == KK - 1),
                                perf_mode=mybir.MatmulPerfMode.DoubleRow,
                            )
                        o_sb = opool.tile([P, NT], y_dt, tag="o")
                        nc.scalar.activation(
                            o_sb[:], ps[:],
                            mybir.ActivationFunctionType.Identity,
                            bias=b_sb[:, m:m + 1],
                            scale=DESCALE,
                        )
                        nc.sync.dma_start(
                            yt[m * P:(m + 1) * P, si * NT:(si + 1) * NT], o_sb[:]
                        )

            if loops == 1:
                body()
            else:
                unroll = int(os.environ.get("CANTOR_BENCH_UNROLL", "1"))
                assert loops % unroll == 0
                with tc.For_i(0, loops // unroll, 1) as i:
                    for _ in range(unroll):
                        body(i)

    nc.compile()
    return nc


FULL_KLISTS = (tuple(range(KS)),) * MB


def _build_nc(mm_mode: str, loops: int, klists=FULL_KLISTS):
    if mm_mode == "fp8dr":
        return _build_nc_fp8dr(loops)
    mm_cast = {
        "f32r": mybir.dt.float32r,
        "f32": mybir.dt.float32,
        "bf16": mybir.dt.bfloat16,
        "fp16": mybir.dt.float16,
        "e3w16": mybir.dt.float8e3,   # x e3m4, w fp16 (mixed-dtype matmul)
        "e3": mybir.dt.float8e3,      # x and w both e3m4
    }[mm_mode]
    io_dt = mm_cast if mm_mode in ("bf16", "fp16", "e3w16", "e3") else mybir.dt.float32
    w_io_dt = mybir.dt.float16 if mm_mode == "e3w16" else io_dt

    y_dt = mybir.dt.float16 if OUT_FP16 else mybir.dt.float32
    nc = bacc.Bacc("TRN2", target_bir_lowering=False, debug=False)
    n_si = S_SH // NT
    xt = nc.dram_tensor("xt", [n_si, P, KS, NT], io_dt, kind="ExternalInput")
    wt = nc.dram_tensor("wt", [IN_F, R_PAD], w_io_dt, kind="ExternalInput")
    bc = nc.dram_tensor("bc", [R_PAD], mybir.dt.float32, kind="ExternalInput")
    yt = nc.dram_tensor("yt", [R_PAD, S_SH], y_dt, kind="ExternalOutput")

    wt_r = wt.rearrange("(ko p) r -> p ko r", p=P)
    bc_r = bc.rearrange("(m p) -> p m", p=P)

    n_si = S_SH // NT
    if LOOP == "kouter":
        xbufs = int(os.environ.get("CANTOR_XBUFS", str(2 * n_si)))
        obufs = int(os.environ.get("CANTOR_OBUFS", str(MB * n_si)))
        psbufs = int(os.environ.get("CANTOR_PSBUFS", str(MB * n_si)))
    else:
        xbufs = int(os.environ.get("CANTOR_XBUFS", "4"))
        obufs = int(os.environ.get("CANTOR_OBUFS", "4"))
        psbufs = int(os.environ.get("CANTOR_PSBUFS", "4"))

    with tile.TileContext(nc) as tc:
        is_f32r = mm_cast == mybir.dt.float32r
        with (
            tc.tile_pool(name="wpool", bufs=1) as wpool,
            tc.tile_pool(name="xpool", bufs=xbufs) as xpool,
            tc.tile_pool(name="opool", bufs=obufs) as opool,
            tc.tile_pool(name="pspool", bufs=psbufs, space="PSUM") as pspool,
        ):
            w_ld = wpool.tile([P, KS, R_PAD], w_io_dt)
            nc.sync.dma_start(w_ld[:], wt_r)
            b_sb = wpool.tile([P, MB], mybir.dt.float32)
            nc.sync.dma_start(b_sb[:], bc_r)
            if is_f32r:
                w_sb = wpool.tile([P, KS, R_PAD], mybir.dt.float32r)
                nc.vector.tensor_copy(w_sb[:], w_ld[:])
            else:
                w_sb = w_ld

            # e3w16 folds the 1/SX3 descale into the fp16 weights host-side.
            desc = 1.0 / (SX3 * SW3) if mm_mode == "e3" else 1.0

            def evict(ps, m, si, engine):
                o_sb = opool.tile([P, NT], y_dt, tag="o")
                if engine == "dve" and desc == 1.0:
                    nc.vector.tensor_tensor(
                        o_sb[:], ps[:],
                        b_sb[:, m:m + 1].to_broadcast([P, NT]),
                        mybir.AluOpType.add,
                    )
                else:
                    nc.scalar.activation(
                        o_sb[:], ps[:],
                        mybir.ActivationFunctionType.Identity,
                        bias=b_sb[:, m:m + 1],
                        scale=desc,
                    )
                nc.sync.dma_start(
                    yt[m * P:(m + 1) * P, si * NT:(si + 1) * NT], o_sb[:]
                )

            def body_siouter(_i=None):
                for si in range(n_si):
                    x_ld = xpool.tile([P, KS, NT], io_dt, tag="xld")
                    nc.sync.dma_start(x_ld[:], xt[si])
                    if is_f32r:
                        x_sb = xpool.tile([P, KS, NT], mybir.dt.float32r, tag="xr")
                        nc.vector.tensor_copy(x_sb[:], x_ld[:])
                    else:
                        x_sb = x_ld
                    for m in range(MB):
                        ps = pspool.tile([P, NT], mybir.dt.float32, tag="ps")
                        kl = klists[m]
                        for k in kl:
                            nc.tensor.matmul(
                                ps[:],
                                lhsT=w_sb[:, k, m * P:(m + 1) * P],
                                rhs=x_sb[:, k, :],
                                start=(k == kl[0]),
                                stop=(k == kl[-1]),
                            )
                        evict(ps, m, si, "act")

            def body_kouter(_i=None):
                xs, pss = [], {}
                for si in range(n_si):
                    x_ld = xpool.tile([P, KS, NT], io_dt, tag="xld")
                    nc.sync.dma_start(x_ld[:], xt[si])
                    xs.append(x_ld)
                for m in range(MB):
                    for si in range(n_si):
                        pss[(m, si)] = pspool.tile(
                            [P, NT], mybir.dt.float32, tag="ps",
                            name=f"ps_{m}_{si}")
                # run k-subtiles shared by both blocks first, exclusive ones
                # last, so the narrower block's PSUMs retire early and their
                # evictions overlap the tail matmuls
                shared = [k for k in range(KS)
                          if all(k in kl for kl in klists)]
                korder = shared + [k for k in range(KS) if k not in shared]
                ex = {m: [k for k in korder if k in klists[m]]
                      for m in range(MB)}
                done = []
                for k in korder:
                    for m in range(MB):
                        if k not in klists[m]:
                            continue
                        for si in range(n_si):
                            nc.tensor.matmul(
                                pss[(m, si)][:],
                                lhsT=w_sb[:, k, m * P:(m + 1) * P],
                                rhs=xs[si][:, k, :],
                                start=(k == ex[m][0]),
                                stop=(k == ex[m][-1]),
                            )
                        if k == ex[m][-1]:
                            for si in range(n_si):
                                done.append((m, si))
                for i, (m, si) in enumerate(done):
                    evict(pss[(m, si)], m, si, "dve" if i % 2 else "act")

            body = body_kouter if LOOP == "kouter" else body_siouter

            if loops == 1:
                body()
            else:
                unroll = int(os.environ.get("CANTOR_BENCH_UNROLL", "1"))
                assert loops % unroll == 0
                with tc.For_i(0, loops // unroll, 1) as i:
                    for _ in range(unroll):
                        body(i)

    nc.compile()
    return nc


LAST_KLISTS = FULL_KLISTS   # set by prep_in_maps, read by _get_nc


def _get_nc(mm_mode: str, loops: int, klists=None):
    if klists is None:
        klists = LAST_KLISTS
    key = (mm_mode, loops, klists)
    if key not in _NC_CACHE:
        _NC_CACHE[key] = _build_nc(mm_mode, loops, klists)
    return _NC_CACHE[key]


def _row_blocks(mask):
    """Sort nonzero rows so the second 128-row block touches as few
    128-wide k-subtiles as possible; returns (row order, per-block k lists).

    The Cantor runs are ~639 wide with sliding starts, so the best 2-block
    split only trims a few subtiles — but those are free to skip."""
    kt = mask.reshape(OUT_F, KS, P).any(axis=2)
    rows = np.flatnonzero(kt.any(axis=1))
    ktr = kt[rows]
    order = np.argsort(ktr.argmax(axis=1), kind="stable")
    # block boundary is fixed at 128 (PSUM partition block); two sort
    # directions, keep the cheaper
    best = None
    for o in (order, order[::-1]):
        ka = tuple(np.flatnonzero(ktr[o[:P]].any(axis=0)).tolist())
        kb = tuple(np.flatnonzero(ktr[o[P:]].any(axis=0)).tolist())
        if best is None or len(ka) + len(kb) < best[0]:
            best = (len(ka) + len(kb), o, (ka, kb))
    _, o, klists = best
    return rows[o], klists


def _pack_k_pairs(a):
    """[R, IN_F] -> [P, KD, 2, R]: k = kk*256 + i*128 + p -> [p, kk, i]."""
    r = a.shape[0]
    return np.ascontiguousarray(
        a.reshape(r, KD, 2, P).transpose(3, 1, 2, 0))


def prep_in_maps(x, weight, bias, mask):
    """Host-side prep: pack compact weight/bias and per-core transposed x
    shards. Returns (in_maps, rows)."""
    import ml_dtypes

    x = np.asarray(x, dtype=np.float32)
    weight = np.asarray(weight, dtype=np.float32)
    bias = np.asarray(bias, dtype=np.float32)
    mask = np.asarray(mask, dtype=np.float32)

    global LAST_KLISTS
    w_eff = weight * mask
    if MM_MODE in ("e3w16", "e3", "fp16", "bf16") and \
            os.environ.get("CANTOR_BSPARSE", "1") == "1":
        rows, klists = _row_blocks(mask != 0)
    else:
        rows = np.flatnonzero(mask.any(axis=1))
        klists = FULL_KLISTS
    LAST_KLISTS = klists
    r = len(rows)
    assert r <= R_PAD, f"compact rows {r} > padded {R_PAD}"

    w_c = np.zeros((R_PAD, IN_F), dtype=np.float32)
    w_c[:r] = w_eff[rows]
    bc = np.zeros((R_PAD,), dtype=np.float32)
    bc[:r] = bias[rows]

    xf = x.reshape(S, IN_F)
    n_si = S_SH // NT
    in_maps = []

    if MM_MODE == "fp8dr":
        f8 = ml_dtypes.float8_e4m3
        w_s = w_c * SW
        w8 = w_s.astype(f8)
        if WCOMP:
            rw8 = (w_s - w8.astype(np.float32)).astype(f8)
            wt = np.concatenate(
                [_pack_k_pairs(w8), _pack_k_pairs(rw8)], axis=1)  # [P, 2*KD, 2, R_PAD]
        else:
            wt = _pack_k_pairs(w8)
        for c in range(NCORES):
            x_t = (xf[c * S_SH:(c + 1) * S_SH].T * SX).astype(f8)  # [IN_F, S_SH]
            # [IN_F, S_SH] -> [KD, 2, P, n_si, NT] -> [n_si, P, KD, 2, NT]
            x_t = np.ascontiguousarray(
                x_t.reshape(KD, 2, P, n_si, NT).transpose(3, 2, 0, 1, 4))
            in_maps.append({"xt": x_t, "wt": wt, "bc": bc})
        return in_maps, rows

    x_scale = 1.0
    if MM_MODE == "bf16":
        io_np = ml_dtypes.bfloat16
        w_np = io_np
    elif MM_MODE == "fp16":
        io_np = np.float16
        w_np = io_np
    elif MM_MODE == "e3w16":
        io_np = ml_dtypes.float8_e3m4
        w_np = np.float16
        x_scale = SX3
        w_c = w_c / SX3    # fold the 1/SX3 descale into the fp16 weights
    elif MM_MODE == "e3":
        io_np = ml_dtypes.float8_e3m4
        w_np = io_np
        x_scale = SX3
        w_c = w_c * SW3
    else:
        io_np = np.float32
        w_np = io_np

    wt = np.ascontiguousarray(w_c.T).astype(w_np)       # [IN_F, R_PAD]
    for c in range(NCORES):
        xs = xf[c * S_SH:(c + 1) * S_SH].T
        if x_scale != 1.0:
            xs = xs * x_scale
        x_t = xs.astype(io_np)                           # one-pass T + cast
        # [IN_F, S_SH] -> [n_si, P, KS, NT]; partition-major contiguous
        x_t = np.ascontiguousarray(
            x_t.reshape(KS, P, n_si, NT).transpose(2, 1, 0, 3))
        in_maps.append({"xt": x_t, "wt": wt, "bc": bc})
    return in_maps, rows


def kernel(x, weight, bias, mask):
    global LAST_RESULTS
    bias = np.asarray(bias, dtype=np.float32)
    in_maps, rows = prep_in_maps(x, weight, bias, mask)
    r = len(rows)

    nc = _get_nc(MM_MODE, LOOPS)
    res = run_bass_kernel_spmd(nc, in_maps, list(range(NCORES)))
    LAST_RESULTS = res

    y = np.empty((S, OUT_F), dtype=np.float32)
    y[:] = bias
    for c in range(NCORES):
        y[c * S_SH:(c + 1) * S_SH, rows] = \
            res.results[c]["yt"][:r].T.astype(np.float32)
    return y.reshape(B, SQ, OUT_F)
